# revision 14
# baseline (speedup 1.0000x reference)
import numpy as np
from contextlib import ExitStack

from concourse import bass, bacc, tile, masks, bass_utils
from concourse.bass import mybir

P = 128          # partitions / rows per block
N = 2048         # rows per core (batch entry)
D = 512          # feature dim
NB = N // P      # 16 row blocks
KC = D // P      # 4 contraction chunks
E = 576          # padded table row (512 F2 + 9 G9 + pad), 2304B % 256 == 0
F32 = mybir.dt.float32
I16 = mybir.dt.int16
U16 = mybir.dt.uint16
AF = mybir.ActivationFunctionType
ALU = mybir.AluOpType
AX = mybir.AxisListType


def _build():
    nc = bacc.Bacc("TRN2")
    feats_d = nc.declare_dram_parameter("feats", [N, D], F32, isOutput=False)
    fcwt_d = nc.declare_dram_parameter("fcwt", [P, KC, D], F32, isOutput=False)
    w9t_d = nc.declare_dram_parameter("w9t", [P, KC, 9], F32, isOutput=False)
    b9_d = nc.declare_dram_parameter("b9", [P, 9], F32, isOutput=False)
    sel_d = nc.declare_dram_parameter("sel", [P, 8 * P], F32, isOutput=False)
    out_d = nc.declare_dram_parameter("out", [N, D], F32, isOutput=True)

    table_d = nc.dram_tensor("table", [N, E], F32)

    with tile.TileContext(nc) as tc, ExitStack() as ctx:
        pers = ctx.enter_context(tc.tile_pool(name="pers", bufs=1))
        xn_all = pers.tile([P, NB * D], F32, name="xn_all")
        xnt = pers.tile([P, KC * N], F32, name="xnt")
        fcwt_sb = pers.tile([P, KC * D], F32, name="fcwt_sb")
        w9t_sb = pers.tile([P, KC * 9], F32, name="w9t_sb")
        b9_sb = pers.tile([P, 9], F32, name="b9_sb")
        sel_sb = pers.tile([P, 8 * P], F32, name="sel_sb")
        norm_all = pers.tile([P, NB], F32, name="norm_all")
        rnorm_all = pers.tile([P, NB], F32, name="rnorm_all")
        ident = pers.tile([P, P], F32, name="ident")

        masks.make_identity(nc, ident[:])
        nc.sync.dma_start(fcwt_sb[:], fcwt_d[:])
        nc.sync.dma_start(w9t_sb[:], w9t_d[:])
        nc.sync.dma_start(b9_sb[:], b9_d[:])
        nc.sync.dma_start(sel_sb[:], sel_d[:])

        # Phase A: load, normalize, transpose, F2/G9 projections -> DRAM table
        with tc.tile_pool(name="sbA", bufs=2) as sbA, \
             tc.tile_pool(name="ppA", bufs=2, space="PSUM") as ppA:
            for c in range(NB):
                nrm = norm_all[:, c:c + 1]
                rnm = rnorm_all[:, c:c + 1]
                f_c = sbA.tile([P, D], F32, name="f_c")
                nc.sync.dma_start(f_c[:], feats_d[c * P:(c + 1) * P, :])
                sq = sbA.tile([P, D], F32, name="sq")
                nc.scalar.activation(sq[:], f_c[:], AF.Square, accum_out=nrm)
                nc.scalar.activation(nrm, nrm, AF.Sqrt)
                nc.vector.tensor_scalar_max(rnm, nrm, 1e-12)
                nc.vector.reciprocal(rnm, rnm)
                xn_c = xn_all[:, c * D:(c + 1) * D]
                nc.scalar.mul(xn_c, f_c[:], rnm)

                pt = ppA.tile([P, KC * P], F32, name="pt")
                for k in range(KC):
                    nc.tensor.transpose(pt[:, k * P:(k + 1) * P],
                                        xn_c[:, k * P:(k + 1) * P], ident[:])
                xnt_view = xnt.rearrange("p (k n) -> p k n", k=KC)[:, :, c * P:(c + 1) * P]
                nc.scalar.copy(xnt_view, pt.rearrange("p (k n) -> p k n", k=KC))

                pf2 = ppA.tile([P, D], F32, name="pf2")
                pg9 = ppA.tile([P, 9], F32, name="pg9")
                for k in range(KC):
                    lhsT = xnt[:, k * N + c * P: k * N + (c + 1) * P]
                    nc.tensor.matmul(pf2[:], lhsT, fcwt_sb[:, k * D:(k + 1) * D],
                                     start=(k == 0), stop=(k == KC - 1))
                for k in range(KC):
                    lhsT = xnt[:, k * N + c * P: k * N + (c + 1) * P]
                    nc.tensor.matmul(pg9[:], lhsT, w9t_sb[:, k * 9:(k + 1) * 9],
                                     start=(k == 0), stop=(k == KC - 1))
                stg = sbA.tile([P, E], F32, name="stg")
                nc.scalar.mul(stg[:, 0:D], pf2[:], nrm)
                nc.scalar.mul(stg[:, D:D + 9], pg9[:], nrm)
                nc.vector.memset(stg[:, D + 9:E], 0.0)
                nc.sync.dma_start(table_d[c * P:(c + 1) * P, :], stg[:])

        # Phase B: sims matmul, top-3, gather, softmax, weighted sum, relu
        with tc.tile_pool(name="sbB", bufs=2) as sbB, \
             tc.tile_pool(name="sbG", bufs=2) as sbG, \
             tc.tile_pool(name="ppB", bufs=2, space="PSUM") as ppB:
            for c in range(NB):
                ps = ppB.tile([P, N], F32, name="ps")
                for j in range(4):
                    for k in range(KC):
                        nc.tensor.matmul(
                            ps[:, j * 512:(j + 1) * 512],
                            xnt[:, k * N + c * P: k * N + (c + 1) * P],
                            xnt[:, k * N + j * 512: k * N + (j + 1) * 512],
                            start=(k == 0), stop=(k == KC - 1))
                vmax = sbB.tile([P, 8], F32, name="vmax")
                idx8 = sbB.tile([P, 8], U16, name="idx8")
                nc.vector.max(vmax[:], ps[:])
                nc.vector.max_index(idx8[:], vmax[:], ps[:])

                # Build the gather-index layout idxs[16r+l, 8g+h] = idx8[16h+l, g]
                # via 8 selection matmuls: Sel_h[p, p'] = (p%16 == p'%16) & (p//16 == h).
                # Indices are exact small integers in fp32; result reuses ps[:, 0:24]
                # (already consumed by max/max_index).
                idxf = sbB.tile([P, 3], F32, name="idxf")
                nc.vector.tensor_copy(idxf[:], idx8[:, 0:3])
                pidx = ps[:, 0:24]
                for h in range(8):
                    nc.tensor.matmul(pidx[:, 3 * h:3 * h + 3],
                                     sel_sb[:, h * P:(h + 1) * P], idxf[:])
                idxs_sb = sbB.tile([P, 24], I16, name="idxs_sb")
                src = bass.AP(pidx.tensor, pidx.offset, [pidx.ap[0], [1, 3], [3, 8]])
                nc.vector.tensor_copy(idxs_sb.rearrange("p (g h) -> p g h", g=3), src)

                gat = sbG.tile([P, 3, E], F32, name="gat")
                nc.gpsimd.dma_gather(gat[:], table_d[:], idxs_sb[:],
                                     num_idxs=3 * P, num_idxs_reg=3 * P, elem_size=E)

                # mult[p, g, j] = G9[idx[p,g], 3g+j] + b[g, j]
                g0 = gat[:, 0:3, D:D + 3]
                diag = bass.AP(g0.tensor, g0.offset, [g0.ap[0], [E + 3, 3], [1, 3]])
                m9 = sbB.tile([P, 3, 3], F32, name="m9")
                nc.vector.tensor_add(m9[:], diag, b9_sb.rearrange("p (g j) -> p g j", g=3))
                e9 = sbB.tile([P, 3, 3], F32, name="e9")
                nc.scalar.activation(e9[:], m9[:], AF.Exp)
                s3 = sbB.tile([P, 3], F32, name="s3")
                nc.vector.tensor_reduce(s3[:], e9[:], AX.X, ALU.add)
                nc.vector.reciprocal(s3[:], s3[:])
                w9s = sbB.tile([P, 3, 3], F32, name="w9s")
                nc.vector.tensor_mul(w9s[:], e9[:],
                                     s3.unsqueeze(2).broadcast_to((P, 3, 3)))
                c3 = sbB.tile([P, 3], F32, name="c3")
                nc.vector.tensor_reduce(c3[:], w9s[:].transpose((0, 2, 1)), AX.X, ALU.add)
                nc.vector.tensor_scalar_mul(c3[:], c3[:], 1.0 / 3.0)

                acc = sbB.tile([P, D], F32, name="acc")
                nc.vector.tensor_scalar_mul(acc[:], gat[:, 0, 0:D], c3[:, 0:1])
                for j in (1, 2):
                    nc.vector.scalar_tensor_tensor(acc[:], gat[:, j, 0:D],
                                                   c3[:, j:j + 1], acc[:],
                                                   ALU.mult, ALU.add)
                outb = sbB.tile([P, D], F32, name="outb")
                nc.scalar.activation(outb[:], acc[:], AF.Relu)
                nc.sync.dma_start(out_d[c * P:(c + 1) * P, :], outb[:])
    nc.compile()
    return nc


def _host_prep(inputs):
    feats = np.asarray(inputs["feats"], np.float32)
    convKK_w = np.asarray(inputs["convKK_w"], np.float32)
    convKK_b = np.asarray(inputs["convKK_b"], np.float32)
    fc_w = np.asarray(inputs["fc_w"], np.float32)
    fcwt = np.ascontiguousarray(fc_w.T.reshape(KC, P, D).transpose(1, 0, 2))
    w9 = convKK_w.reshape(3 * 3, D)
    w9t = np.ascontiguousarray(w9.T.reshape(KC, P, 9).transpose(1, 0, 2))
    b9 = np.ascontiguousarray(np.tile(convKK_b.reshape(1, 9), (P, 1)))
    p = np.arange(P)
    sel = ((p[:, None] % 16 == p[None, :] % 16)[:, None, :]
           & (p[:, None] // 16 == np.arange(8)[None, :])[:, :, None])
    sel = np.ascontiguousarray(sel.reshape(P, 8 * P).astype(np.float32))
    return feats, fcwt, w9t, b9, sel


def kernel(**inputs):
    feats, fcwt, w9t, b9, sel = _host_prep(inputs)
    b = feats.shape[0]
    nc = _build()
    in_maps = [{"feats": np.ascontiguousarray(feats[i]),
                "fcwt": fcwt, "w9t": w9t, "b9": b9, "sel": sel} for i in range(b)]
    res = bass_utils.run_bass_kernel_spmd(nc, in_maps, core_ids=list(range(b)))
    return np.stack([r["out"] for r in res.results]).astype(np.float32)


# revision 16
# speedup vs baseline: 7102.9487x; 7102.9487x over previous
import numpy as np
from contextlib import ExitStack

from concourse import bass, bacc, tile, masks, bass_utils
from concourse.bass import mybir

P = 128          # partitions / rows per block
N = 2048         # rows per core (batch entry)
D = 512          # feature dim
NB = N // P      # 16 row blocks
KC = D // P      # 4 contraction chunks
E = 576          # padded table row (512 F2 + 9 G9 + pad), 2304B % 256 == 0
F32 = mybir.dt.float32
BF16 = mybir.dt.bfloat16
I16 = mybir.dt.int16
U16 = mybir.dt.uint16
AF = mybir.ActivationFunctionType
ALU = mybir.AluOpType
AX = mybir.AxisListType


def _build():
    nc = bacc.Bacc("TRN2")
    feats_d = nc.declare_dram_parameter("feats", [N, D], F32, isOutput=False)
    fcwt_d = nc.declare_dram_parameter("fcwt", [P, KC, D], F32, isOutput=False)
    w9t_d = nc.declare_dram_parameter("w9t", [P, KC, 9], F32, isOutput=False)
    b9_d = nc.declare_dram_parameter("b9", [P, 9], F32, isOutput=False)
    sel_d = nc.declare_dram_parameter("sel", [P, 8 * P], F32, isOutput=False)
    out_d = nc.declare_dram_parameter("out", [N, D], F32, isOutput=True)

    table_d = nc.dram_tensor("table", [N, E], F32)

    with tile.TileContext(nc) as tc, ExitStack() as ctx:
        pers = ctx.enter_context(tc.tile_pool(name="pers", bufs=1))
        ht = pers.tile([P, KC * N], BF16, name="ht")
        lt = pers.tile([P, KC * N], BF16, name="lt")
        table_sb = pers.tile([P, NB * E], F32, name="table_sb")
        fcwt_sb = pers.tile([P, KC * D], F32, name="fcwt_sb")
        w9t_sb = pers.tile([P, KC * 9], F32, name="w9t_sb")
        b9_sb = pers.tile([P, 9], F32, name="b9_sb")
        sel_sb = pers.tile([P, 8 * P], F32, name="sel_sb")
        norm_all = pers.tile([P, NB], F32, name="norm_all")
        rnorm_all = pers.tile([P, NB], F32, name="rnorm_all")
        ident = pers.tile([P, P], F32, name="ident")

        masks.make_identity(nc, ident[:])
        nc.sync.dma_start(fcwt_sb[:], fcwt_d[:])
        nc.sync.dma_start(w9t_sb[:], w9t_d[:])
        nc.sync.dma_start(b9_sb[:], b9_d[:])
        nc.sync.dma_start(sel_sb[:], sel_d[:])

        # Phase A: load, normalize, transpose, HL-bf16 split, F2/G9 projections
        with tc.tile_pool(name="sbA", bufs=2) as sbA, \
             tc.tile_pool(name="ppA", bufs=2, space="PSUM") as ppA:
            for c in range(NB):
                nrm = norm_all[:, c:c + 1]
                rnm = rnorm_all[:, c:c + 1]
                f_c = sbA.tile([P, D], F32, name="f_c")
                nc.sync.dma_start(f_c[:], feats_d[c * P:(c + 1) * P, :])
                sq = sbA.tile([P, D], F32, name="sq")
                nc.scalar.activation(sq[:], f_c[:], AF.Square, accum_out=nrm)
                nc.scalar.activation(nrm, nrm, AF.Sqrt)
                nc.vector.tensor_scalar_max(rnm, nrm, 1e-12)
                nc.vector.reciprocal(rnm, rnm)
                xn_c = sbA.tile([P, D], F32, name="xn_c")
                nc.scalar.mul(xn_c[:], f_c[:], rnm)

                pt = ppA.tile([P, KC * P], F32, name="pt")
                for k in range(KC):
                    nc.tensor.transpose(pt[:, k * P:(k + 1) * P],
                                        xn_c[:, k * P:(k + 1) * P], ident[:])
                xt_c = sbA.tile([P, KC * P], F32, name="xt_c")
                nc.scalar.copy(xt_c[:], pt[:])

                # bf16 high/low split into the persistent transposed tensors
                xt_r = xt_c.rearrange("p (k i) -> p k i", k=KC)
                ht_v = bass.AP(ht.tensor, ht.offset + c * P,
                               [ht.ap[0], [N, KC], [1, P]])
                lt_v = bass.AP(lt.tensor, lt.offset + c * P,
                               [lt.ap[0], [N, KC], [1, P]])
                nc.vector.tensor_copy(ht_v, xt_r)
                nc.vector.tensor_sub(lt_v, xt_r, ht_v)

                pf2 = ppA.tile([P, D], F32, name="pf2")
                pg9 = ppA.tile([P, 9], F32, name="pg9")
                for k in range(KC):
                    nc.tensor.matmul(pf2[:], xt_c[:, k * P:(k + 1) * P],
                                     fcwt_sb[:, k * D:(k + 1) * D],
                                     start=(k == 0), stop=(k == KC - 1))
                for k in range(KC):
                    nc.tensor.matmul(pg9[:], xt_c[:, k * P:(k + 1) * P],
                                     w9t_sb[:, k * 9:(k + 1) * 9],
                                     start=(k == 0), stop=(k == KC - 1))
                nc.scalar.mul(table_sb[:, c * E:c * E + D], pf2[:], nrm)
                nc.scalar.mul(table_sb[:, c * E + D:c * E + D + 9], pg9[:], nrm)
                nc.vector.memset(table_sb[:, c * E + D + 9:(c + 1) * E], 0.0)
                nc.sync.dma_start(table_d[c * P:(c + 1) * P, :],
                                  table_sb[:, c * E:(c + 1) * E])

        # Phase B: sims (HL-bf16 3-pass), top-3, gather ranks 1-2, epilogue
        with tc.tile_pool(name="sbB", bufs=2) as sbB, \
             tc.tile_pool(name="sbG", bufs=2) as sbG, \
             tc.tile_pool(name="ppB", bufs=2, space="PSUM") as ppB:
            for c in range(NB):
                ps = ppB.tile([P, N], F32, name="ps")
                passes = ((ht, ht), (ht, lt), (lt, ht))
                for pi, (A, B) in enumerate(passes):
                    for k in range(KC):
                        lhsT = A[:, k * N + c * P: k * N + (c + 1) * P]
                        for j in range(4):
                            nc.tensor.matmul(
                                ps[:, j * 512:(j + 1) * 512], lhsT,
                                B[:, k * N + j * 512: k * N + (j + 1) * 512],
                                start=(pi == 0 and k == 0),
                                stop=(pi == 2 and k == KC - 1))
                vmax = sbB.tile([P, 8], F32, name="vmax")
                idx8 = sbB.tile([P, 8], U16, name="idx8")
                nc.vector.max(vmax[:], ps[:])
                nc.vector.max_index(idx8[:], vmax[:], ps[:])

                # Rank 0 is always self (row c*P+p); gather only ranks 1-2.
                # idxs[16r+l, 8g+h] = idx8[16h+l, 1+g] via 8 selection
                # matmuls (exact small ints in fp32), reusing ps[:, 0:16].
                idxf = sbB.tile([P, 2], F32, name="idxf")
                nc.vector.tensor_copy(idxf[:], idx8[:, 1:3])
                pidx = ps[:, 0:16]
                for h in range(8):
                    nc.tensor.matmul(pidx[:, 2 * h:2 * h + 2],
                                     sel_sb[:, h * P:(h + 1) * P], idxf[:])
                idxs_sb = sbB.tile([P, 16], I16, name="idxs_sb")
                src = bass.AP(pidx.tensor, pidx.offset, [pidx.ap[0], [1, 2], [2, 8]])
                nc.vector.tensor_copy(idxs_sb.rearrange("p (g h) -> p g h", g=2), src)

                gat = sbG.tile([P, 2, E], F32, name="gat")
                nc.gpsimd.dma_gather(gat[:], table_d[:], idxs_sb[:],
                                     num_idxs=2 * P, num_idxs_reg=2 * P, elem_size=E)

                # mult[p, g, j] = G9[idx[p,g], 3g+j] + b[g, j]; g=0 from SBUF
                m9 = sbB.tile([P, 3, 3], F32, name="m9")
                nc.vector.tensor_add(m9[:, 0, :],
                                     table_sb[:, c * E + D:c * E + D + 3],
                                     b9_sb[:, 0:3])
                g0 = gat[:, 0:2, D + 3:D + 6]
                diag = bass.AP(g0.tensor, g0.offset, [g0.ap[0], [E + 3, 2], [1, 3]])
                nc.vector.tensor_add(m9[:, 1:3, :], diag,
                                     b9_sb[:, 3:9].rearrange("p (g j) -> p g j", g=2))
                e9 = sbB.tile([P, 3, 3], F32, name="e9")
                nc.scalar.activation(e9[:], m9[:], AF.Exp)
                s3 = sbB.tile([P, 3], F32, name="s3")
                nc.vector.tensor_reduce(s3[:], e9[:], AX.X, ALU.add)
                nc.vector.reciprocal(s3[:], s3[:])
                w9s = sbB.tile([P, 3, 3], F32, name="w9s")
                nc.vector.tensor_mul(w9s[:], e9[:],
                                     s3.unsqueeze(2).broadcast_to((P, 3, 3)))
                c3 = sbB.tile([P, 3], F32, name="c3")
                nc.vector.tensor_reduce(c3[:], w9s[:].transpose((0, 2, 1)), AX.X, ALU.add)
                nc.vector.tensor_scalar_mul(c3[:], c3[:], 1.0 / 3.0)

                acc = sbB.tile([P, D], F32, name="acc")
                nc.vector.tensor_scalar_mul(acc[:], table_sb[:, c * E:c * E + D],
                                            c3[:, 0:1])
                for j in (1, 2):
                    nc.vector.scalar_tensor_tensor(acc[:], gat[:, j - 1, 0:D],
                                                   c3[:, j:j + 1], acc[:],
                                                   ALU.mult, ALU.add)
                outb = sbB.tile([P, D], F32, name="outb")
                nc.scalar.activation(outb[:], acc[:], AF.Relu)
                nc.sync.dma_start(out_d[c * P:(c + 1) * P, :], outb[:])
    nc.compile()
    return nc


def _host_prep(inputs):
    feats = np.asarray(inputs["feats"], np.float32)
    convKK_w = np.asarray(inputs["convKK_w"], np.float32)
    convKK_b = np.asarray(inputs["convKK_b"], np.float32)
    fc_w = np.asarray(inputs["fc_w"], np.float32)
    fcwt = np.ascontiguousarray(fc_w.T.reshape(KC, P, D).transpose(1, 0, 2))
    w9 = convKK_w.reshape(3 * 3, D)
    w9t = np.ascontiguousarray(w9.T.reshape(KC, P, 9).transpose(1, 0, 2))
    b9 = np.ascontiguousarray(np.tile(convKK_b.reshape(1, 9), (P, 1)))
    p = np.arange(P)
    sel = ((p[:, None] % 16 == p[None, :] % 16)[:, None, :]
           & (p[:, None] // 16 == np.arange(8)[None, :])[:, :, None])
    sel = np.ascontiguousarray(sel.reshape(P, 8 * P).astype(np.float32))
    return feats, fcwt, w9t, b9, sel


def kernel(**inputs):
    feats, fcwt, w9t, b9, sel = _host_prep(inputs)
    b = feats.shape[0]
    nc = _build()
    in_maps = [{"feats": np.ascontiguousarray(feats[i]),
                "fcwt": fcwt, "w9t": w9t, "b9": b9, "sel": sel} for i in range(b)]
    res = bass_utils.run_bass_kernel_spmd(nc, in_maps, core_ids=list(range(b)))
    return np.stack([r["out"] for r in res.results]).astype(np.float32)
